# revision 51
# baseline (speedup 1.0000x reference)
"""Distributed Trainium2 kernel for gated RoPE attention (2x2048x1024, 16 heads).

Sharding: 8 cores = 2 batches x 4 head-groups (4 heads each).

v2 restructure vs baseline:
  - gates projection col-tiled (M=4 x 4 concurrent qt tiles) instead of 16
    full-width M=1 matmul groups: ~27us -> ~2us of PE time
  - ss (sum-of-squares) col-tiled the same way: 6.8us -> ~1.7us
  - scores use K=64 row-tiled concurrent matmul pairs (heads A/B in rows
    0-63 / 64-127 of the packed q/k tiles) writing the two halves of one
    [128, 1024] psum tile; one exp per (qt, kc) covers both heads.  This
    halves score-matmul PE time and kills the zero-padded kTz tiles.
  - RoPE pair-swap matmuls quadrant-tiled (block-diag pswap): 2x concurrent
  - x^2 squares moved from ACT to DVE (ACT is the exp bottleneck)
  - the whole pt=1 projection pipeline (QK proj, RoPE, V, gates) is emitted
    interleaved into the pt=0 SDPA loop so the PE consumes it during exp
    stalls; pt=0 gating is interleaved into the pt=1 SDPA loop.
Host sums the 4 per-batch partials (the tensor-parallel reduce).
"""

import sys

for _p in ("/opt/trn_rl_repo",):
    if _p not in sys.path:
        sys.path.insert(0, _p)

import numpy as np
import ml_dtypes

import concourse.bass as bass
import concourse.mybir as mybir
import concourse.tile as tile
from concourse import bacc
from concourse.bass_utils import run_bass_kernel_spmd

BF16 = mybir.dt.bfloat16
F32 = mybir.dt.float32
AF = mybir.ActivationFunctionType

DIM = 1024
HEADS = 16
DH = 64
B = 2
N = 2048
NH = 4          # heads per core
NCORES = 8
P = 128
DC = DIM // P   # 8 contraction chunks
QT = 512        # q tile (free dim per matmul)
WQ = 516        # q(256) | k(256) | gates(4)
WARMKEEPER = True


def build_graph(n=N, dbg=False):
    nc = bacc.Bacc("TRN2", target_bir_lowering=False, debug=False,
                   enable_asserts=False)

    nqt = n // QT       # 4 q tiles
    nkc = n // P        # 16 k chunks
    nnt = n // P        # 16 n chunks (rows of out)

    xT_d = nc.dram_tensor("xT", [DIM, n], BF16, kind="ExternalInput")
    wqkg_d = nc.dram_tensor("w_qkg", [DIM, WQ], BF16, kind="ExternalInput")
    wvp_d = nc.dram_tensor("w_vp", [DIM, NH * 65], BF16, kind="ExternalInput")
    wout_d = nc.dram_tensor("w_out_s", [NH * DH, DIM], BF16, kind="ExternalInput")
    cos_d = nc.dram_tensor("cos_t", [P, n], BF16, kind="ExternalInput")
    sin_d = nc.dram_tensor("sin_t", [P, n], BF16, kind="ExternalInput")
    pswap_d = nc.dram_tensor("pswapT", [P, P], BF16, kind="ExternalInput")
    onesc_d = nc.dram_tensor("ones_col", [P, 1], BF16, kind="ExternalInput")
    onesrb_d = nc.dram_tensor("ones_rowb", [1, P], BF16, kind="ExternalInput")
    bgT_d = nc.dram_tensor("bgT", [NH, 1], F32, kind="ExternalInput")
    out_d = nc.dram_tensor("out", [n, DIM], BF16, kind="ExternalOutput")
    if dbg:
        dbg_rstd = nc.dram_tensor("dbg_rstd", [1, n], F32, kind="ExternalOutput")
        dbg_qk = nc.dram_tensor("dbg_qk", [P, 4 * n], BF16, kind="ExternalOutput")
        dbg_g4 = nc.dram_tensor("dbg_g4", [NH, n], F32, kind="ExternalOutput")
        dbg_vaug = nc.dram_tensor("dbg_vaug", [P, nkc * NH * 65], BF16,
                                  kind="ExternalOutput")
        dbg_oTs0 = nc.dram_tensor("dbg_oTs0", [P, n], BF16, kind="ExternalOutput")
        dbg_smh0 = nc.dram_tensor("dbg_smh0", [DH + 1, n], F32,
                                  kind="ExternalOutput")

    with tile.TileContext(nc) as tc:
        with tc.tile_pool(name="consts", bufs=1) as pc, \
             tc.tile_pool(name="big", bufs=1) as pb, \
             tc.tile_pool(name="work", bufs=2) as pw, \
             tc.tile_pool(name="dram", bufs=1, space="DRAM") as pd, \
             tc.tile_pool(name="probs", bufs=4) as pprob, \
             tc.tile_pool(name="psum", bufs=2, space="PSUM") as ps:

            # ---- inputs to SBUF.  dma_start issue is serialized (~1us each
            # on the sync queue) while transfers stripe across the 16 HW
            # engines, so: few big DMAs, in consumption order (x chunk 0 +
            # its weights first so the first matmuls start ~2-3us in), and
            # the ones-constants are memset instead of DMA'd.
            onesc = pc.tile([P, 1], BF16, tag="onesc", name="onesc")
            nc.gpsimd.memset(onesc[:], 1.0)
            onesrb = pc.tile([1, P], BF16, tag="onesrb", name="onesrb")
            nc.gpsimd.memset(onesrb[:], 1.0)
            # x chunks lead their weight chunks by 2 so the squares (rstd
            # critical path) and the first QK matmuls both start early
            wqkg = pc.tile([P, DC * WQ], BF16, tag="wqkg", name="wqkg")
            xT = pb.tile([P, DC * n], BF16, tag="xT", name="xT")
            for dc in range(DC + 2):
                if dc < DC:
                    nc.sync.dma_start(xT[:, dc * n:(dc + 1) * n],
                                      xT_d[dc * P:(dc + 1) * P, :])
                if dc >= 2:
                    wc = dc - 2
                    nc.sync.dma_start(wqkg[:, wc * WQ:(wc + 1) * WQ],
                                      wqkg_d[wc * P:(wc + 1) * P, :])
            wvp = pc.tile([P, DC * NH * 65], BF16, tag="wvp", name="wvp")
            for dc in range(DC):
                nc.sync.dma_start(wvp[:, dc * NH * 65:(dc + 1) * NH * 65],
                                  wvp_d[dc * P:(dc + 1) * P, :])
            pswap = pc.tile([P, P], BF16, tag="pswap", name="pswap")
            nc.sync.dma_start(pswap[:], pswap_d[:])
            bgT = pc.tile([NH, 1], F32, tag="bgT", name="bgT")
            nc.sync.dma_start(bgT[:], bgT_d[:])
            cos_t = pc.tile([P, n], BF16, tag="cos", name="cos")
            sin_t = pc.tile([P, n], BF16, tag="sin", name="sin")
            nc.sync.dma_start(cos_t[:], cos_d[:])
            nc.sync.dma_start(sin_t[:], sin_d[:])
            wout = pc.tile([P, 2 * DIM], BF16, tag="wout", name="wout")
            for ec in range(2):
                nc.sync.dma_start(wout[:, ec * DIM:(ec + 1) * DIM],
                                  wout_d[ec * P:(ec + 1) * P, :])

            # persistent SBUF tensors
            qkT = [pb.tile([P, n], BF16, tag=f"qkT{i}", name=f"qkT{i}")
                   for i in range(4)]
            rstd = pb.tile([1, n], F32, tag="rstd", name="rstd")
            rstd_b = pb.tile([P, n], BF16, tag="rstdb", name="rstdb")
            rstd_p = pb.tile([P, n // P], F32, tag="rstdp", name="rstdp")
            vaug = pb.tile([P, nkc * NH * 65], BF16, tag="vaug", name="vaug")
            oTs = [pb.tile([P, n], BF16, tag=f"oTs{i}", name=f"oTs{i}")
                   for i in range(2)]
            g4 = pb.tile([NH, n], F32, tag="g4", name="g4")
            # packed row-vector tiles: heads 2i / 2i+1 at partitions 0 / 64
            gsh2 = [pb.tile([DH + 1, n], F32, tag=f"gsh{i}", name=f"gsh{i}")
                    for i in range(2)]
            smh2 = [pb.tile([DH + 1, n], F32, tag=f"smh{i}", name=f"smh{i}")
                    for i in range(2)]
            for _t in smh2:
                nc.gpsimd.memset(_t[:], 1.0)

            def gsh(h):
                return gsh2[h // 2][(h % 2) * DH:(h % 2) * DH + 1, :]

            def smh(h):
                return smh2[h // 2][(h % 2) * DH:(h % 2) * DH + 1, :]

            # ================= prologue =================
            # -- stage B: ss = sum_d x^2 (col-tiled concurrent M=1 pairs;
            # qt 2j / 2j+1 at partition bases 0 / 64 of tile j) --
            ss2 = [ps.tile([DH + 1, QT], F32, tag="po", name=f"ss{j}")
                   for j in range(2)]
            for dc in range(DC):
                x2 = pw.tile([P, n], BF16, tag="x2", name="x2")
                # alternate squares between ACT and DVE so the last chunk's
                # square (the rstd critical path) isn't ACT-serialized
                if dc % 2 == 0:
                    nc.scalar.activation(x2[:], xT[:, dc * n:(dc + 1) * n],
                                         AF.Square)
                else:
                    nc.vector.tensor_mul(x2[:], xT[:, dc * n:(dc + 1) * n],
                                         xT[:, dc * n:(dc + 1) * n])
                for qt in range(nqt):
                    nc.tensor.matmul(
                        ss2[qt // 2][(qt % 2) * DH:(qt % 2) * DH + 1, :],
                        onesc[:], x2[:, qt * QT:(qt + 1) * QT],
                        start=(dc == 0), stop=(dc == DC - 1),
                        skip_group_check=True)
            for qt in range(nqt):
                sq = pw.tile([1, QT], F32, tag="sq", name="sq")
                nc.scalar.sqrt(sq[:],
                               ss2[qt // 2][(qt % 2) * DH:(qt % 2) * DH + 1, :])
                nc.vector.reciprocal_approx_fast(
                    rstd[0:1, qt * QT:(qt + 1) * QT], sq[:])
            # broadcast rstd across partitions (PE, K=1, bf16 operands)
            rstdb16 = pw.tile([1, n], BF16, tag="rstdb16", name="rstdb16",
                              bufs=1)
            nc.vector.tensor_copy(rstdb16[:], rstd[:])
            for qt in range(nqt):
                bp = ps.tile([P, QT], F32, tag="bg", name="bc")
                nc.tensor.matmul(bp[:], onesrb[:],
                                 rstdb16[0:1, qt * QT:(qt + 1) * QT],
                                 start=True, stop=True)
                nc.vector.tensor_copy(rstd_b[:, qt * QT:(qt + 1) * QT], bp[:])
            # rstd in [n-partition, chunk] layout via DRAM round-trip
            scr = pd.tile([1, n], F32, tag="scr", name="scr")
            nc.sync.dma_start(scr[0:1, :], rstd[0:1, :])
            nc.sync.dma_start(
                rstd_p[:],
                scr[0:1, :].rearrange("o (c p) -> (o p) c", p=P))
            # cos/sin tables with the rstd token scale folded in (rstd
            # commutes with RoPE's within-token rotation)
            cosr = pb.tile([P, n], BF16, tag="cosr", name="cosr")
            sinr = pb.tile([P, n], BF16, tag="sinr", name="sinr")
            nc.vector.tensor_mul(cosr[:], cos_t[:], rstd_b[:])
            nc.vector.tensor_mul(sinr[:], sin_t[:], rstd_b[:])

            # -- QK projection for one packed tile (2 heads).  The rstd
            # per-token scale commutes with RoPE (a per-column scale), so it
            # is folded into the cos/sin tables instead; the PSUM drain here
            # is a plain copy, routed to ACT (idle during the prologue) --
            def qk_proj(et, pair, qts=None):
                step = 2 if pair else 1
                tag = "sc" if pair else "bg"
                if qts is None:
                    qts = range(0, nqt, step)
                for q0 in qts:
                    pp = ps.tile([P, step * QT], F32, tag=tag, name="pp")
                    for dc in range(DC):
                        for j in range(step):
                            qt = q0 + j
                            nc.tensor.matmul(
                                pp[:, j * QT:(j + 1) * QT],
                                wqkg[:, dc * WQ + et * 128:
                                     dc * WQ + et * 128 + 128],
                                xT[:, dc * n + qt * QT:dc * n + (qt + 1) * QT],
                                start=(dc == 0), stop=(dc == DC - 1),
                                skip_group_check=True)
                            yield 213
                    for j in range(step):
                        qt = q0 + j
                        sl = slice(qt * QT, (qt + 1) * QT)
                        if pair:
                            nc.scalar.copy(qkT[et][:, sl],
                                           pp[:, j * QT:(j + 1) * QT])
                        else:
                            nc.vector.tensor_copy(qkT[et][:, sl],
                                                  pp[:, j * QT:(j + 1) * QT])
                        yield 0

            # -- gates: col-tiled M=4 matmuls, all 4 heads at once.  Runs as
            # background inside the pt0 SDPA loop, so the sigmoid is computed
            # as 0.5*tanh(z/2)+0.5 (tanh shares the exp ACT table -- no
            # mid-stream table reload; bgT holds b_gates/2 host-side) --
            def gates_proj():
                pg2 = [ps.tile([DH + NH, QT], F32, tag="bg", name=f"pg{j}")
                       for j in range(2)]
                for dc in range(DC):
                    for qt in range(nqt):
                        rb = (qt % 2) * DH
                        nc.tensor.matmul(
                            pg2[qt // 2][rb:rb + NH, :],
                            wqkg[:, dc * WQ + 512:dc * WQ + 516],
                            xT[:, dc * n + qt * QT:dc * n + (qt + 1) * QT],
                            start=(dc == 0), stop=(dc == DC - 1),
                            skip_group_check=True)
                    yield 250
                for qt in range(nqt):
                    sl = slice(qt * QT, (qt + 1) * QT)
                    rb = (qt % 2) * DH
                    nc.vector.tensor_mul(g4[0:NH, sl],
                                         pg2[qt // 2][rb:rb + NH, :],
                                         rstd_b[rb:rb + NH, sl])
                    yield 0
                nc.scalar.activation(g4[:], g4[:], AF.Tanh, scale=0.5,
                                     bias=bgT[:])
                nc.vector.tensor_scalar(g4[:], g4[:], 0.5, 0.5,
                                        mybir.AluOpType.mult,
                                        mybir.AluOpType.add)
                yield 0
                # scatter head rows into the packed gsh2 tiles via DRAM
                scr4 = pd.tile([NH, n], F32, tag="scr4", name="scr4")
                nc.sync.dma_start(scr4[:], g4[:])
                for h in range(NH):
                    nc.sync.dma_start(gsh(h), scr4[h:h + 1, :])
                yield 0
                if dbg:
                    nc.sync.dma_start(dbg_g4[:], g4[:])
                    yield 0

            # -- RoPE on one packed tile, in place (quadrant-tiled pswap).
            # cosr/sinr have the rstd token scale folded in.
            def rope(pt, qts=None):
                if qts is None:
                    qts = range(nqt)
                for qt in qts:
                    sl = slice(qt * QT, (qt + 1) * QT)
                    t1 = pw.tile([P, QT], BF16, tag="ropec", name="t1")
                    nc.vector.tensor_mul(t1[:], qkT[pt][:, sl], cosr[:, sl])
                    qks = pw.tile([P, QT], BF16, tag="ropes", name="qks")
                    nc.vector.tensor_mul(qks[:], qkT[pt][:, sl], sinr[:, sl])
                    pr = ps.tile([P, QT], F32, tag="bg", name="pr")
                    nc.tensor.matmul(pr[0:DH, :], pswap[0:DH, 0:DH],
                                     qks[0:DH, :],
                                     start=True, stop=True,
                                     skip_group_check=True)
                    nc.tensor.matmul(pr[DH:P, :], pswap[DH:P, DH:P],
                                     qks[DH:P, :],
                                     start=True, stop=True,
                                     skip_group_check=True)
                    yield 230
                    nc.vector.tensor_add(qkT[pt][:, sl], t1[:], pr[:])
                    yield 0

            # -- V projection for one k-chunk, all 4 heads at once (amortizes
            # the x-chunk LDWEIGHTS over the full 260-col moving pass).
            # Chunks are emitted interleaved into the first SDPA q-tile's
            # kc loop (chunk kc+2 at iteration kc) so the exp stream starts
            # as soon as RoPE is done instead of after a 14us V block. --
            def v_chunk(kc):
                pv = ps.tile([P, NH * 65], F32, tag="bg", name="pv")
                for dc in range(DC):
                    nc.tensor.matmul(
                        pv[:],
                        xT[:, dc * n + kc * P:dc * n + (kc + 1) * P],
                        wvp[:, dc * NH * 65:(dc + 1) * NH * 65],
                        start=(dc == 0), stop=(dc == DC - 1),
                        skip_group_check=True)
                vsl = slice(kc * NH * 65, (kc + 1) * NH * 65)
                nc.vector.tensor_scalar_mul(vaug[:, vsl], pv[:],
                                            rstd_p[:, kc:kc + 1])
                nc.gpsimd.memset(vaug[:, kc * NH * 65 + DH::65], 1.0)

            # -- gating for a head pair (after its SDPA sums are complete) --
            def gate_whole(i):
                rec = pw.tile([DH + 1, n], F32, tag="recw", name="rec", bufs=1)
                nc.vector.reciprocal_approx_fast(rec[:], smh2[i][:])
                yield 0
                for h in (2 * i, 2 * i + 1):
                    rb = (h % 2) * DH
                    ft = pw.tile([1, n], BF16, tag="ftw", name="ft", bufs=2)
                    nc.vector.tensor_mul(ft[:], rec[rb:rb + 1, :], gsh(h))
                    yield 0
                    for qt in range(nqt):
                        qsl = slice(qt * QT, (qt + 1) * QT)
                        pf = ps.tile([DH, QT], F32, tag="bg", name="pf")
                        nc.tensor.matmul(pf[:], onesrb[0:1, 0:DH],
                                         ft[0:1, qsl], start=True, stop=True)
                        nc.vector.tensor_mul(oTs[i][rb:rb + DH, qsl],
                                             oTs[i][rb:rb + DH, qsl], pf[:])
                        yield 213

            # single-qt slice of the gating chain (tail pipelining)
            def gate_slice(i, qt):
                qsl = slice(qt * QT, (qt + 1) * QT)
                rec = pw.tile([DH + 1, QT], F32, tag="recs", name="rec")
                nc.vector.reciprocal_approx_fast(rec[:], smh2[i][:, qsl])
                yield 0
                for h in (2 * i, 2 * i + 1):
                    rb = (h % 2) * DH
                    ft = pw.tile([1, QT], BF16, tag="fts", name="ft")
                    nc.vector.tensor_mul(ft[:], rec[rb:rb + 1, :],
                                         gsh2[i][rb:rb + 1, qsl])
                    pf = ps.tile([DH, QT], F32, tag="bg", name="pf")
                    nc.tensor.matmul(pf[:], onesrb[0:1, 0:DH], ft[0:1, :],
                                     start=True, stop=True)
                    nc.vector.tensor_mul(oTs[i][rb:rb + DH, qsl],
                                         oTs[i][rb:rb + DH, qsl], pf[:])
                    yield 213

            # one n-chunk of the output projection (contraction over both
            # packed oTs tiles), DMA'd out as soon as it is built
            def out_nt(nt):
                ob = pw.tile([P, DIM], BF16, tag="ob", name="ob")
                for dh in range(2):
                    pp2 = ps.tile([P, QT], F32, tag="bg", name="pp2")
                    for ec in range(2):
                        nc.tensor.matmul(
                            pp2[:],
                            oTs[ec][:, nt * P:(nt + 1) * P],
                            wout[:, ec * DIM + dh * QT:
                                 ec * DIM + dh * QT + QT],
                            start=(ec == 0), stop=(ec == 1))
                        yield 213
                    if dh == 0:
                        nc.vector.tensor_copy(ob[:, dh * QT:(dh + 1) * QT],
                                              pp2[:])
                    else:
                        nc.scalar.copy(ob[:, dh * QT:(dh + 1) * QT], pp2[:])
                    yield 0
                nc.sync.dma_start(out_d[nt * P:(nt + 1) * P, :], ob[:])
                yield 0

            # run a generator to completion immediately
            def run_now(gen):
                for _ in gen:
                    pass

            def chain(*gens):
                for g in gens:
                    for c in g:
                        yield c

            # prologue: only what the first score matmuls need -- all of k
            # (qk2/rope2), the first q tile, and V.  Everything else (gates,
            # q tiles 1-3, the pt1 pipeline) drains inside the SDPA loops.
            run_now(qk_proj(2, pair=True))
            run_now(qk_proj(0, pair=True, qts=[0]))
            run_now(rope(2))
            run_now(rope(0, qts=[0]))
            v_chunk(0)
            v_chunk(1)

            if dbg:
                nc.sync.dma_start(dbg_rstd[:], rstd[:])

            # ================= SDPA =================
            # background generators drained during the SDPA loops
            # bgA: work the later pt0 q-tiles depend on (forced complete at
            # the qt0 boundary); bgB: the pt1 pipeline (credit-drained)
            bgA = chain(rope(0, qts=[1]), gates_proj(),
                        qk_proj(0, pair=False, qts=[2, 3]),
                        rope(0, qts=[2, 3]))
            bgB = chain(qk_proj(1, pair=False), qk_proj(3, pair=False),
                        rope(1), rope(3))
            bg2 = chain(gate_whole(0))
            # NOTE: generators must only be advanced in a phase where their
            # data dependencies' producers have already been EMITTED --
            # emission order defines program order (the gating of oTs[0] must
            # not be emitted before sdpa(0)'s epilogue writes).
            state = {"credit": 0, "gens": [bgA, bgB]}

            def drain(budget):
                state["credit"] += budget
                gens = state["gens"]
                while gens and state["credit"] > 0:
                    try:
                        state["credit"] -= next(gens[0])
                    except StopIteration:
                        gens.pop(0)

            def sdpa(pt, post_qt=None):
                # heads 2pt (rows 0:64) and 2pt+1 (rows 64:128)
                voff = pt * 2 * 65
                for qt in range(nqt):
                    qsl = slice(qt * QT, (qt + 1) * QT)
                    pos = [ps.tile([DH + 1, QT], F32, tag="po", name=f"po{e}")
                           for e in range(2)]
                    for kc in range(nkc):
                        ksl = slice(kc * P, (kc + 1) * P)
                        sc = ps.tile([P, 2 * QT], F32, tag="sc", name="sc")
                        if pt == 1 and WARMKEEPER and kc % 4 == 0:
                            # warm-keeper: pt1 has little background work, so
                            # PE micro-idles re-throttle the HAM clock to
                            # 1.2GHz and cold matmuls then gate the exp
                            # stream.  A dummy matmul per iteration (into the
                            # region the scores overwrite with start=True)
                            # keeps the PE dense and the clock at 2.4GHz.
                            nc.tensor.matmul(sc[:, 0:QT],
                                             qkT[2 + pt][:, ksl],
                                             qkT[pt][:, qsl],
                                             start=True, stop=True,
                                             skip_group_check=True)
                        nc.tensor.matmul(sc[:, 0:QT],
                                         qkT[2 + pt][0:DH, ksl],
                                         qkT[pt][0:DH, qsl],
                                         start=True, stop=True,
                                         skip_group_check=True)
                        nc.tensor.matmul(sc[:, QT:2 * QT],
                                         qkT[2 + pt][DH:P, ksl],
                                         qkT[pt][DH:P, qsl],
                                         start=True, stop=True,
                                         skip_group_check=True)
                        pr = pprob.tile([P, 2 * QT], BF16, tag="pr", name="pr")
                        nc.scalar.activation(pr[:], sc[:], AF.Exp,
                                             scale=float(DH) ** -0.5)
                        for e in range(2):
                            nc.tensor.matmul(
                                pos[e][:],
                                vaug[:, kc * NH * 65 + voff + e * 65:
                                     kc * NH * 65 + voff + (e + 1) * 65],
                                pr[:, e * QT:(e + 1) * QT],
                                start=(kc == 0), stop=(kc == nkc - 1),
                                skip_group_check=True)
                        if pt == 0 and qt == 0:
                            # stream the remaining V chunks two iterations
                            # ahead of their PV consumer; no drain here (the
                            # PE is already oversubscribed in this q tile)
                            if kc + 2 < nkc:
                                v_chunk(kc + 2)
                        else:
                            drain(356)
                    for e in range(2):
                        h = 2 * pt + e
                        rb = e * DH
                        nc.vector.tensor_copy(oTs[pt][rb:rb + DH, qsl],
                                              pos[e][0:DH, :])
                        nc.vector.tensor_copy(smh(h)[0:1, qsl],
                                              pos[e][DH:DH + 1, :])
                    if post_qt is not None:
                        post_qt(qt)

            def post_qt0(qt):
                if qt == 0:
                    # later q tiles' projections/rope must be emitted before
                    # their score matmuls (emission order defines deps)
                    run_now(bgA)

            sdpa(0, post_qt=post_qt0)
            # anything not yet drained must be emitted before pt1 sdpa
            run_now(bgA)
            run_now(bgB)
            state["gens"] = [bg2]

            def post_qt1(qt):
                # pt0 gating must be fully emitted before out_nt reads oTs[0]
                run_now(bg2)
                # queue this qt's gating + its 4 output chunks; drained by
                # the remaining SDPA iterations (all forced at the end)
                state["gens"].append(chain(gate_slice(1, qt),
                                           *[out_nt(nt) for nt in
                                             range(4 * qt, 4 * qt + 4)]))

            sdpa(1, post_qt=post_qt1)
            for g in state["gens"]:
                run_now(g)
            state["gens"] = []

            if dbg:
                nc.sync.dma_start(dbg_qk[:, 0:n], qkT[0][:])
                nc.sync.dma_start(dbg_qk[:, n:2 * n], qkT[1][:])
                nc.sync.dma_start(dbg_qk[:, 2 * n:3 * n], qkT[2][:])
                nc.sync.dma_start(dbg_qk[:, 3 * n:4 * n], qkT[3][:])
                nc.sync.dma_start(dbg_vaug[:], vaug[:])
                nc.sync.dma_start(dbg_oTs0[:], oTs[0][:])
                nc.sync.dma_start(dbg_smh0[:], smh2[0][:])

    nc.compile()
    return nc


def host_prep(x, gamma, w_qkv, w_gates, b_gates, w_out, freqs, n=N):
    """Build the 8 per-core input maps (numpy, host-side)."""
    x = np.asarray(x, dtype=np.float32)
    gamma = np.asarray(gamma, dtype=np.float32)
    w_qkv = np.asarray(w_qkv, dtype=np.float32)
    w_gates = np.asarray(w_gates, dtype=np.float32)
    b_gates = np.asarray(b_gates, dtype=np.float32)
    w_out = np.asarray(w_out, dtype=np.float32)
    freqs = np.asarray(freqs, dtype=np.float32)

    bf = ml_dtypes.bfloat16
    gvec = gamma * (DIM ** 0.5)

    pos = np.arange(n, dtype=np.float32)
    ang = pos[:, None] * freqs[None, :]          # [n, 32]
    idx = (np.arange(P) % DH) // 2               # row -> freq index
    cos_t = np.cos(ang)[:, idx].T.astype(bf)     # [128, n]
    sin_t = np.sin(ang)[:, idx].T.astype(bf)

    PT = np.zeros((DH, DH), dtype=np.float32)
    for i in range(DH // 2):
        PT[2 * i + 1, 2 * i] = -1.0
        PT[2 * i, 2 * i + 1] = 1.0
    pswapT = np.zeros((P, P), dtype=np.float32)
    pswapT[0:DH, 0:DH] = PT
    pswapT[DH:P, DH:P] = PT
    pswapT = pswapT.astype(bf)

    ones_col = np.ones((P, 1), dtype=bf)
    ones_rowb = np.ones((1, P), dtype=bf)

    in_maps = []
    for c in range(NCORES):
        bi, hg = divmod(c, 4)
        hs = hg * NH
        xT = np.ascontiguousarray(x[bi, :n].T).astype(bf)
        wq = w_qkv[:, hs * DH:(hs + NH) * DH]
        wk = w_qkv[:, HEADS * DH + hs * DH:HEADS * DH + (hs + NH) * DH]
        wv = w_qkv[:, 2 * HEADS * DH + hs * DH:2 * HEADS * DH + (hs + NH) * DH]
        wg = w_gates[:, hs:hs + NH]
        w_qkg = (np.concatenate([wq, wk, wg], axis=1)
                 * gvec[:, None]).astype(bf)
        w_vp = np.zeros((DIM, NH * 65), dtype=np.float32)
        for h in range(NH):
            w_vp[:, h * 65:h * 65 + DH] = wv[:, h * DH:(h + 1) * DH]
        w_vp = (w_vp * gvec[:, None]).astype(bf)
        w_out_s = w_out[hs * DH:(hs + NH) * DH, :].astype(bf)
        # halved: the kernel computes sigmoid(z+b) as 0.5*tanh((z+b)/2)+0.5
        bgT = (b_gates[hs:hs + NH] / 2.0).reshape(NH, 1).astype(np.float32)
        in_maps.append({
            "xT": xT, "w_qkg": w_qkg, "w_vp": w_vp, "w_out_s": w_out_s,
            "cos_t": cos_t, "sin_t": sin_t, "pswapT": pswapT,
            "ones_col": ones_col, "ones_rowb": ones_rowb, "bgT": bgT,
        })
    return in_maps


_NC_CACHE = {}


def _ensure_ntff_hook():
    """antenv.axon_hooks is missing on this image; recreate it and register
    the ctypes NTFF profiling hook from trn_agent_boot so trace=True works."""
    try:
        from antenv.axon_hooks import get_axon_ntff_profile_hook  # noqa: F401
        return
    except ImportError:
        pass
    import types
    try:
        import antenv
    except ImportError:
        return
    mod = types.ModuleType("antenv.axon_hooks")
    holder = {}
    mod.set_axon_ntff_profile_hook = lambda h: holder.__setitem__("h", h)
    mod.get_axon_ntff_profile_hook = lambda: holder.get("h")
    sys.modules["antenv.axon_hooks"] = mod
    antenv.axon_hooks = mod
    try:
        from trn_agent_boot.trn_boot import _ntff_profile_via_ctypes
        h = _ntff_profile_via_ctypes("/opt/axon/libaxon_pjrt.so")
        if h is not None:
            mod.set_axon_ntff_profile_hook(h)
    except Exception:
        pass


def run(inputs, trace=False, n=N, dbg=False):
    if trace:
        _ensure_ntff_hook()
    key = (n, dbg)
    if key not in _NC_CACHE:
        _NC_CACHE[key] = build_graph(n, dbg=dbg)
    nc = _NC_CACHE[key]
    in_maps = host_prep(**inputs, n=n)
    kw = {}
    if trace:
        kw = dict(trace=True, trace_cores=[0])
    res = run_bass_kernel_spmd(nc, in_maps, core_ids=list(range(NCORES)), **kw)
    parts = [np.asarray(r["out"], dtype=np.float32) for r in res.results]
    out = np.stack([
        parts[0] + parts[1] + parts[2] + parts[3],
        parts[4] + parts[5] + parts[6] + parts[7],
    ]).astype(np.float32)
    return out, res


def kernel(**inputs):
    out, _ = run(inputs, trace=False)
    return out



# revision 52
# speedup vs baseline: 1.0339x; 1.0339x over previous
"""Distributed Trainium2 kernel for gated RoPE attention (2x2048x1024, 16 heads).

Sharding: 8 cores = 2 batches x 4 head-groups (4 heads each).

v2 restructure vs baseline:
  - gates projection col-tiled (M=4 x 4 concurrent qt tiles) instead of 16
    full-width M=1 matmul groups: ~27us -> ~2us of PE time
  - ss (sum-of-squares) col-tiled the same way: 6.8us -> ~1.7us
  - scores use K=64 row-tiled concurrent matmul pairs (heads A/B in rows
    0-63 / 64-127 of the packed q/k tiles) writing the two halves of one
    [128, 1024] psum tile; one exp per (qt, kc) covers both heads.  This
    halves score-matmul PE time and kills the zero-padded kTz tiles.
  - RoPE pair-swap matmuls quadrant-tiled (block-diag pswap): 2x concurrent
  - x^2 squares moved from ACT to DVE (ACT is the exp bottleneck)
  - the whole pt=1 projection pipeline (QK proj, RoPE, V, gates) is emitted
    interleaved into the pt=0 SDPA loop so the PE consumes it during exp
    stalls; pt=0 gating is interleaved into the pt=1 SDPA loop.
Host sums the 4 per-batch partials (the tensor-parallel reduce).
"""

import sys

for _p in ("/opt/trn_rl_repo",):
    if _p not in sys.path:
        sys.path.insert(0, _p)

import numpy as np
import ml_dtypes

import concourse.bass as bass
import concourse.mybir as mybir
import concourse.tile as tile
from concourse import bacc
from concourse.bass_utils import run_bass_kernel_spmd

BF16 = mybir.dt.bfloat16
F32 = mybir.dt.float32
AF = mybir.ActivationFunctionType

DIM = 1024
HEADS = 16
DH = 64
B = 2
N = 2048
NH = 4          # heads per core
NCORES = 8
P = 128
DC = DIM // P   # 8 contraction chunks
QT = 512        # q tile (free dim per matmul)
WQ = 516        # q(256) | k(256) | gates(4)
WARMKEEPER = False


def build_graph(n=N, dbg=False):
    nc = bacc.Bacc("TRN2", target_bir_lowering=False, debug=False,
                   enable_asserts=False)

    nqt = n // QT       # 4 q tiles
    nkc = n // P        # 16 k chunks
    nnt = n // P        # 16 n chunks (rows of out)

    xT_d = nc.dram_tensor("xT", [DIM, n], BF16, kind="ExternalInput")
    wqkg_d = nc.dram_tensor("w_qkg", [DIM, WQ], BF16, kind="ExternalInput")
    wvp_d = nc.dram_tensor("w_vp", [DIM, NH * 65], BF16, kind="ExternalInput")
    wout_d = nc.dram_tensor("w_out_s", [NH * DH, DIM], BF16, kind="ExternalInput")
    cos_d = nc.dram_tensor("cos_t", [P, n], BF16, kind="ExternalInput")
    sin_d = nc.dram_tensor("sin_t", [P, n], BF16, kind="ExternalInput")
    pswap_d = nc.dram_tensor("pswapT", [P, P], BF16, kind="ExternalInput")
    onesc_d = nc.dram_tensor("ones_col", [P, 1], BF16, kind="ExternalInput")
    onesrb_d = nc.dram_tensor("ones_rowb", [1, P], BF16, kind="ExternalInput")
    bgT_d = nc.dram_tensor("bgT", [NH, 1], F32, kind="ExternalInput")
    out_d = nc.dram_tensor("out", [n, DIM], BF16, kind="ExternalOutput")
    if dbg:
        dbg_rstd = nc.dram_tensor("dbg_rstd", [1, n], F32, kind="ExternalOutput")
        dbg_qk = nc.dram_tensor("dbg_qk", [P, 4 * n], BF16, kind="ExternalOutput")
        dbg_g4 = nc.dram_tensor("dbg_g4", [NH, n], F32, kind="ExternalOutput")
        dbg_vaug = nc.dram_tensor("dbg_vaug", [P, nkc * NH * 65], BF16,
                                  kind="ExternalOutput")
        dbg_oTs0 = nc.dram_tensor("dbg_oTs0", [P, n], BF16, kind="ExternalOutput")
        dbg_smh0 = nc.dram_tensor("dbg_smh0", [DH + 1, n], F32,
                                  kind="ExternalOutput")

    with tile.TileContext(nc) as tc:
        with tc.tile_pool(name="consts", bufs=1) as pc, \
             tc.tile_pool(name="big", bufs=1) as pb, \
             tc.tile_pool(name="work", bufs=2) as pw, \
             tc.tile_pool(name="dram", bufs=1, space="DRAM") as pd, \
             tc.tile_pool(name="probs", bufs=4) as pprob, \
             tc.tile_pool(name="psum", bufs=2, space="PSUM") as ps:

            # ---- inputs to SBUF.  dma_start issue is serialized (~1us each
            # on the sync queue) while transfers stripe across the 16 HW
            # engines, so: few big DMAs, in consumption order (x chunk 0 +
            # its weights first so the first matmuls start ~2-3us in), and
            # the ones-constants are memset instead of DMA'd.
            onesc = pc.tile([P, 1], BF16, tag="onesc", name="onesc")
            nc.gpsimd.memset(onesc[:], 1.0)
            onesrb = pc.tile([1, P], BF16, tag="onesrb", name="onesrb")
            nc.gpsimd.memset(onesrb[:], 1.0)
            # x chunks lead their weight chunks by 2 so the squares (rstd
            # critical path) and the first QK matmuls both start early
            wqkg = pc.tile([P, DC * WQ], BF16, tag="wqkg", name="wqkg")
            xT = pb.tile([P, DC * n], BF16, tag="xT", name="xT")
            for dc in range(DC + 2):
                if dc < DC:
                    nc.sync.dma_start(xT[:, dc * n:(dc + 1) * n],
                                      xT_d[dc * P:(dc + 1) * P, :])
                if dc >= 2:
                    wc = dc - 2
                    nc.sync.dma_start(wqkg[:, wc * WQ:(wc + 1) * WQ],
                                      wqkg_d[wc * P:(wc + 1) * P, :])
            wvp = pc.tile([P, DC * NH * 65], BF16, tag="wvp", name="wvp")
            for dc in range(DC):
                nc.sync.dma_start(wvp[:, dc * NH * 65:(dc + 1) * NH * 65],
                                  wvp_d[dc * P:(dc + 1) * P, :])
            pswap = pc.tile([P, P], BF16, tag="pswap", name="pswap")
            nc.sync.dma_start(pswap[:], pswap_d[:])
            bgT = pc.tile([NH, 1], F32, tag="bgT", name="bgT")
            nc.sync.dma_start(bgT[:], bgT_d[:])
            cos_t = pc.tile([P, n], BF16, tag="cos", name="cos")
            sin_t = pc.tile([P, n], BF16, tag="sin", name="sin")
            nc.sync.dma_start(cos_t[:], cos_d[:])
            nc.sync.dma_start(sin_t[:], sin_d[:])
            wout = pc.tile([P, 2 * DIM], BF16, tag="wout", name="wout")
            for ec in range(2):
                nc.sync.dma_start(wout[:, ec * DIM:(ec + 1) * DIM],
                                  wout_d[ec * P:(ec + 1) * P, :])

            # persistent SBUF tensors
            qkT = [pb.tile([P, n], BF16, tag=f"qkT{i}", name=f"qkT{i}")
                   for i in range(4)]
            rstd = pb.tile([1, n], F32, tag="rstd", name="rstd")
            rstd_b = pb.tile([P, n], BF16, tag="rstdb", name="rstdb")
            rstd_p = pb.tile([P, n // P], F32, tag="rstdp", name="rstdp")
            vaug = pb.tile([P, nkc * NH * 65], BF16, tag="vaug", name="vaug")
            oTs = [pb.tile([P, n], BF16, tag=f"oTs{i}", name=f"oTs{i}")
                   for i in range(2)]
            g4 = pb.tile([NH, n], F32, tag="g4", name="g4")
            # packed row-vector tiles: heads 2i / 2i+1 at partitions 0 / 64
            gsh2 = [pb.tile([DH + 1, n], F32, tag=f"gsh{i}", name=f"gsh{i}")
                    for i in range(2)]
            smh2 = [pb.tile([DH + 1, n], F32, tag=f"smh{i}", name=f"smh{i}")
                    for i in range(2)]
            for _t in smh2:
                nc.gpsimd.memset(_t[:], 1.0)

            def gsh(h):
                return gsh2[h // 2][(h % 2) * DH:(h % 2) * DH + 1, :]

            def smh(h):
                return smh2[h // 2][(h % 2) * DH:(h % 2) * DH + 1, :]

            # ================= prologue =================
            # -- stage B: ss = sum_d x^2 (col-tiled concurrent M=1 pairs;
            # qt 2j / 2j+1 at partition bases 0 / 64 of tile j) --
            ss2 = [ps.tile([DH + 1, QT], F32, tag="po", name=f"ss{j}")
                   for j in range(2)]
            for dc in range(DC):
                x2 = pw.tile([P, n], BF16, tag="x2", name="x2")
                # alternate squares between ACT and DVE so the last chunk's
                # square (the rstd critical path) isn't ACT-serialized
                if dc % 2 == 0:
                    nc.scalar.activation(x2[:], xT[:, dc * n:(dc + 1) * n],
                                         AF.Square)
                else:
                    nc.vector.tensor_mul(x2[:], xT[:, dc * n:(dc + 1) * n],
                                         xT[:, dc * n:(dc + 1) * n])
                for qt in range(nqt):
                    nc.tensor.matmul(
                        ss2[qt // 2][(qt % 2) * DH:(qt % 2) * DH + 1, :],
                        onesc[:], x2[:, qt * QT:(qt + 1) * QT],
                        start=(dc == 0), stop=(dc == DC - 1),
                        skip_group_check=True)
            for qt in range(nqt):
                sq = pw.tile([1, QT], F32, tag="sq", name="sq")
                nc.scalar.sqrt(sq[:],
                               ss2[qt // 2][(qt % 2) * DH:(qt % 2) * DH + 1, :])
                nc.vector.reciprocal_approx_fast(
                    rstd[0:1, qt * QT:(qt + 1) * QT], sq[:])
            # broadcast rstd across partitions (PE, K=1, bf16 operands)
            rstdb16 = pw.tile([1, n], BF16, tag="rstdb16", name="rstdb16",
                              bufs=1)
            nc.vector.tensor_copy(rstdb16[:], rstd[:])
            for qt in range(nqt):
                bp = ps.tile([P, QT], F32, tag="bg", name="bc")
                nc.tensor.matmul(bp[:], onesrb[:],
                                 rstdb16[0:1, qt * QT:(qt + 1) * QT],
                                 start=True, stop=True)
                nc.vector.tensor_copy(rstd_b[:, qt * QT:(qt + 1) * QT], bp[:])
            # rstd in [n-partition, chunk] layout via DRAM round-trip
            scr = pd.tile([1, n], F32, tag="scr", name="scr")
            nc.sync.dma_start(scr[0:1, :], rstd[0:1, :])
            nc.sync.dma_start(
                rstd_p[:],
                scr[0:1, :].rearrange("o (c p) -> (o p) c", p=P))
            # cos/sin tables with the rstd token scale folded in (rstd
            # commutes with RoPE's within-token rotation)
            cosr = pb.tile([P, n], BF16, tag="cosr", name="cosr")
            sinr = pb.tile([P, n], BF16, tag="sinr", name="sinr")
            nc.vector.tensor_mul(cosr[:], cos_t[:], rstd_b[:])
            nc.vector.tensor_mul(sinr[:], sin_t[:], rstd_b[:])

            # -- QK projection for one packed tile (2 heads).  The rstd
            # per-token scale commutes with RoPE (a per-column scale), so it
            # is folded into the cos/sin tables instead; the PSUM drain here
            # is a plain copy, routed to ACT (idle during the prologue) --
            def qk_proj(et, pair, qts=None):
                step = 2 if pair else 1
                tag = "sc" if pair else "bg"
                if qts is None:
                    qts = range(0, nqt, step)
                for q0 in qts:
                    pp = ps.tile([P, step * QT], F32, tag=tag, name="pp")
                    for dc in range(DC):
                        for j in range(step):
                            qt = q0 + j
                            nc.tensor.matmul(
                                pp[:, j * QT:(j + 1) * QT],
                                wqkg[:, dc * WQ + et * 128:
                                     dc * WQ + et * 128 + 128],
                                xT[:, dc * n + qt * QT:dc * n + (qt + 1) * QT],
                                start=(dc == 0), stop=(dc == DC - 1),
                                skip_group_check=True)
                            yield 213
                    for j in range(step):
                        qt = q0 + j
                        sl = slice(qt * QT, (qt + 1) * QT)
                        if pair:
                            nc.scalar.copy(qkT[et][:, sl],
                                           pp[:, j * QT:(j + 1) * QT])
                        else:
                            nc.vector.tensor_copy(qkT[et][:, sl],
                                                  pp[:, j * QT:(j + 1) * QT])
                        yield 0

            # -- gates: col-tiled M=4 matmuls, all 4 heads at once.  Runs as
            # background inside the pt0 SDPA loop, so the sigmoid is computed
            # as 0.5*tanh(z/2)+0.5 (tanh shares the exp ACT table -- no
            # mid-stream table reload; bgT holds b_gates/2 host-side) --
            def gates_proj():
                pg2 = [ps.tile([DH + NH, QT], F32, tag="bg", name=f"pg{j}")
                       for j in range(2)]
                for dc in range(DC):
                    for qt in range(nqt):
                        rb = (qt % 2) * DH
                        nc.tensor.matmul(
                            pg2[qt // 2][rb:rb + NH, :],
                            wqkg[:, dc * WQ + 512:dc * WQ + 516],
                            xT[:, dc * n + qt * QT:dc * n + (qt + 1) * QT],
                            start=(dc == 0), stop=(dc == DC - 1),
                            skip_group_check=True)
                    yield 250
                for qt in range(nqt):
                    sl = slice(qt * QT, (qt + 1) * QT)
                    rb = (qt % 2) * DH
                    nc.vector.tensor_mul(g4[0:NH, sl],
                                         pg2[qt // 2][rb:rb + NH, :],
                                         rstd_b[rb:rb + NH, sl])
                    yield 0
                nc.scalar.activation(g4[:], g4[:], AF.Tanh, scale=0.5,
                                     bias=bgT[:])
                nc.vector.tensor_scalar(g4[:], g4[:], 0.5, 0.5,
                                        mybir.AluOpType.mult,
                                        mybir.AluOpType.add)
                yield 0
                # scatter head rows into the packed gsh2 tiles via DRAM
                scr4 = pd.tile([NH, n], F32, tag="scr4", name="scr4")
                nc.sync.dma_start(scr4[:], g4[:])
                for h in range(NH):
                    nc.sync.dma_start(gsh(h), scr4[h:h + 1, :])
                yield 0
                if dbg:
                    nc.sync.dma_start(dbg_g4[:], g4[:])
                    yield 0

            # -- RoPE on one packed tile, in place (quadrant-tiled pswap).
            # cosr/sinr have the rstd token scale folded in.
            def rope(pt, qts=None):
                if qts is None:
                    qts = range(nqt)
                for qt in qts:
                    sl = slice(qt * QT, (qt + 1) * QT)
                    t1 = pw.tile([P, QT], BF16, tag="ropec", name="t1")
                    nc.vector.tensor_mul(t1[:], qkT[pt][:, sl], cosr[:, sl])
                    qks = pw.tile([P, QT], BF16, tag="ropes", name="qks")
                    nc.vector.tensor_mul(qks[:], qkT[pt][:, sl], sinr[:, sl])
                    pr = ps.tile([P, QT], F32, tag="bg", name="pr")
                    nc.tensor.matmul(pr[0:DH, :], pswap[0:DH, 0:DH],
                                     qks[0:DH, :],
                                     start=True, stop=True,
                                     skip_group_check=True)
                    nc.tensor.matmul(pr[DH:P, :], pswap[DH:P, DH:P],
                                     qks[DH:P, :],
                                     start=True, stop=True,
                                     skip_group_check=True)
                    yield 230
                    nc.vector.tensor_add(qkT[pt][:, sl], t1[:], pr[:])
                    yield 0

            # -- V projection for one k-chunk, all 4 heads at once (amortizes
            # the x-chunk LDWEIGHTS over the full 260-col moving pass).
            # Chunks are emitted interleaved into the first SDPA q-tile's
            # kc loop (chunk kc+2 at iteration kc) so the exp stream starts
            # as soon as RoPE is done instead of after a 14us V block. --
            def v_chunk(kc):
                pv = ps.tile([P, NH * 65], F32, tag="bg", name="pv")
                for dc in range(DC):
                    nc.tensor.matmul(
                        pv[:],
                        xT[:, dc * n + kc * P:dc * n + (kc + 1) * P],
                        wvp[:, dc * NH * 65:(dc + 1) * NH * 65],
                        start=(dc == 0), stop=(dc == DC - 1),
                        skip_group_check=True)
                vsl = slice(kc * NH * 65, (kc + 1) * NH * 65)
                nc.vector.tensor_scalar_mul(vaug[:, vsl], pv[:],
                                            rstd_p[:, kc:kc + 1])
                nc.gpsimd.memset(vaug[:, kc * NH * 65 + DH::65], 1.0)

            # -- gating for a head pair (after its SDPA sums are complete) --
            def gate_whole(i):
                rec = pw.tile([DH + 1, n], F32, tag="recw", name="rec", bufs=1)
                nc.vector.reciprocal_approx_fast(rec[:], smh2[i][:])
                yield 0
                for h in (2 * i, 2 * i + 1):
                    rb = (h % 2) * DH
                    ft = pw.tile([1, n], BF16, tag="ftw", name="ft", bufs=2)
                    nc.vector.tensor_mul(ft[:], rec[rb:rb + 1, :], gsh(h))
                    yield 0
                    for qt in range(nqt):
                        qsl = slice(qt * QT, (qt + 1) * QT)
                        pf = ps.tile([DH, QT], F32, tag="bg", name="pf")
                        nc.tensor.matmul(pf[:], onesrb[0:1, 0:DH],
                                         ft[0:1, qsl], start=True, stop=True)
                        nc.vector.tensor_mul(oTs[i][rb:rb + DH, qsl],
                                             oTs[i][rb:rb + DH, qsl], pf[:])
                        yield 213

            # single-qt slice of the gating chain (tail pipelining)
            def gate_slice(i, qt):
                qsl = slice(qt * QT, (qt + 1) * QT)
                rec = pw.tile([DH + 1, QT], F32, tag="recs", name="rec")
                nc.vector.reciprocal_approx_fast(rec[:], smh2[i][:, qsl])
                yield 0
                for h in (2 * i, 2 * i + 1):
                    rb = (h % 2) * DH
                    ft = pw.tile([1, QT], BF16, tag="fts", name="ft")
                    nc.vector.tensor_mul(ft[:], rec[rb:rb + 1, :],
                                         gsh2[i][rb:rb + 1, qsl])
                    pf = ps.tile([DH, QT], F32, tag="bg", name="pf")
                    nc.tensor.matmul(pf[:], onesrb[0:1, 0:DH], ft[0:1, :],
                                     start=True, stop=True)
                    nc.vector.tensor_mul(oTs[i][rb:rb + DH, qsl],
                                         oTs[i][rb:rb + DH, qsl], pf[:])
                    yield 213

            # one n-chunk of the output projection (contraction over both
            # packed oTs tiles), DMA'd out as soon as it is built
            def out_nt(nt):
                ob = pw.tile([P, DIM], BF16, tag="ob", name="ob")
                for dh in range(2):
                    pp2 = ps.tile([P, QT], F32, tag="bg", name="pp2")
                    for ec in range(2):
                        nc.tensor.matmul(
                            pp2[:],
                            oTs[ec][:, nt * P:(nt + 1) * P],
                            wout[:, ec * DIM + dh * QT:
                                 ec * DIM + dh * QT + QT],
                            start=(ec == 0), stop=(ec == 1))
                        yield 213
                    if dh == 0:
                        nc.vector.tensor_copy(ob[:, dh * QT:(dh + 1) * QT],
                                              pp2[:])
                    else:
                        nc.scalar.copy(ob[:, dh * QT:(dh + 1) * QT], pp2[:])
                    yield 0
                nc.sync.dma_start(out_d[nt * P:(nt + 1) * P, :], ob[:])
                yield 0

            # run a generator to completion immediately
            def run_now(gen):
                for _ in gen:
                    pass

            def chain(*gens):
                for g in gens:
                    for c in g:
                        yield c

            # prologue: only what the first score matmuls need -- all of k
            # (qk2/rope2), the first q tile, and V.  Everything else (gates,
            # q tiles 1-3, the pt1 pipeline) drains inside the SDPA loops.
            run_now(qk_proj(2, pair=True))
            run_now(qk_proj(0, pair=True, qts=[0]))
            run_now(rope(2))
            run_now(rope(0, qts=[0]))
            v_chunk(0)
            v_chunk(1)

            if dbg:
                nc.sync.dma_start(dbg_rstd[:], rstd[:])

            # ================= SDPA =================
            # background generators drained during the SDPA loops
            # bgA: work the later pt0 q-tiles depend on (forced complete at
            # the qt0 boundary); bgB: the pt1 pipeline (credit-drained)
            bgA = chain(rope(0, qts=[1]), gates_proj(),
                        qk_proj(0, pair=False, qts=[2, 3]),
                        rope(0, qts=[2, 3]))
            bgB = chain(qk_proj(1, pair=False), qk_proj(3, pair=False),
                        rope(1), rope(3))
            bg2 = chain(gate_whole(0))
            # NOTE: generators must only be advanced in a phase where their
            # data dependencies' producers have already been EMITTED --
            # emission order defines program order (the gating of oTs[0] must
            # not be emitted before sdpa(0)'s epilogue writes).
            state = {"credit": 0, "gens": [bgA, bgB]}

            def drain(budget):
                state["credit"] += budget
                gens = state["gens"]
                while gens and state["credit"] > 0:
                    try:
                        state["credit"] -= next(gens[0])
                    except StopIteration:
                        gens.pop(0)

            def sdpa(pt, post_qt=None):
                # heads 2pt (rows 0:64) and 2pt+1 (rows 64:128)
                voff = pt * 2 * 65
                for qt in range(nqt):
                    qsl = slice(qt * QT, (qt + 1) * QT)
                    pos = [ps.tile([DH + 1, QT], F32, tag="po", name=f"po{e}")
                           for e in range(2)]
                    for kc in range(nkc):
                        ksl = slice(kc * P, (kc + 1) * P)
                        sc = ps.tile([P, 2 * QT], F32, tag="sc", name="sc")
                        if pt == 1 and WARMKEEPER:
                            # warm-keeper: pt1 has little background work, so
                            # PE micro-idles re-throttle the HAM clock to
                            # 1.2GHz and cold matmuls then gate the exp
                            # stream.  A dummy matmul per iteration (into the
                            # region the scores overwrite with start=True)
                            # keeps the PE dense and the clock at 2.4GHz.
                            nc.tensor.matmul(sc[:, 0:QT],
                                             qkT[2 + pt][:, ksl],
                                             qkT[pt][:, qsl],
                                             start=True, stop=True,
                                             skip_group_check=True)
                        nc.tensor.matmul(sc[:, 0:QT],
                                         qkT[2 + pt][0:DH, ksl],
                                         qkT[pt][0:DH, qsl],
                                         start=True, stop=True,
                                         skip_group_check=True)
                        nc.tensor.matmul(sc[:, QT:2 * QT],
                                         qkT[2 + pt][DH:P, ksl],
                                         qkT[pt][DH:P, qsl],
                                         start=True, stop=True,
                                         skip_group_check=True)
                        pr = pprob.tile([P, 2 * QT], BF16, tag="pr", name="pr")
                        nc.scalar.activation(pr[:], sc[:], AF.Exp,
                                             scale=float(DH) ** -0.5)
                        for e in range(2):
                            nc.tensor.matmul(
                                pos[e][:],
                                vaug[:, kc * NH * 65 + voff + e * 65:
                                     kc * NH * 65 + voff + (e + 1) * 65],
                                pr[:, e * QT:(e + 1) * QT],
                                start=(kc == 0), stop=(kc == nkc - 1),
                                skip_group_check=True)
                        if pt == 0 and qt == 0:
                            # stream the remaining V chunks two iterations
                            # ahead of their PV consumer; no drain here (the
                            # PE is already oversubscribed in this q tile)
                            if kc + 2 < nkc:
                                v_chunk(kc + 2)
                        else:
                            drain(356)
                    for e in range(2):
                        h = 2 * pt + e
                        rb = e * DH
                        nc.vector.tensor_copy(oTs[pt][rb:rb + DH, qsl],
                                              pos[e][0:DH, :])
                        nc.vector.tensor_copy(smh(h)[0:1, qsl],
                                              pos[e][DH:DH + 1, :])
                    if post_qt is not None:
                        post_qt(qt)

            def post_qt0(qt):
                if qt == 0:
                    # later q tiles' projections/rope must be emitted before
                    # their score matmuls (emission order defines deps)
                    run_now(bgA)

            sdpa(0, post_qt=post_qt0)
            # anything not yet drained must be emitted before pt1 sdpa
            run_now(bgA)
            run_now(bgB)
            state["gens"] = [bg2]

            def post_qt1(qt):
                # pt0 gating must be fully emitted before out_nt reads oTs[0]
                run_now(bg2)
                # queue this qt's gating + its 4 output chunks; drained by
                # the remaining SDPA iterations (all forced at the end)
                state["gens"].append(chain(gate_slice(1, qt),
                                           *[out_nt(nt) for nt in
                                             range(4 * qt, 4 * qt + 4)]))

            sdpa(1, post_qt=post_qt1)
            for g in state["gens"]:
                run_now(g)
            state["gens"] = []

            if dbg:
                nc.sync.dma_start(dbg_qk[:, 0:n], qkT[0][:])
                nc.sync.dma_start(dbg_qk[:, n:2 * n], qkT[1][:])
                nc.sync.dma_start(dbg_qk[:, 2 * n:3 * n], qkT[2][:])
                nc.sync.dma_start(dbg_qk[:, 3 * n:4 * n], qkT[3][:])
                nc.sync.dma_start(dbg_vaug[:], vaug[:])
                nc.sync.dma_start(dbg_oTs0[:], oTs[0][:])
                nc.sync.dma_start(dbg_smh0[:], smh2[0][:])

    nc.compile()
    return nc


def host_prep(x, gamma, w_qkv, w_gates, b_gates, w_out, freqs, n=N):
    """Build the 8 per-core input maps (numpy, host-side)."""
    x = np.asarray(x, dtype=np.float32)
    gamma = np.asarray(gamma, dtype=np.float32)
    w_qkv = np.asarray(w_qkv, dtype=np.float32)
    w_gates = np.asarray(w_gates, dtype=np.float32)
    b_gates = np.asarray(b_gates, dtype=np.float32)
    w_out = np.asarray(w_out, dtype=np.float32)
    freqs = np.asarray(freqs, dtype=np.float32)

    bf = ml_dtypes.bfloat16
    gvec = gamma * (DIM ** 0.5)

    pos = np.arange(n, dtype=np.float32)
    ang = pos[:, None] * freqs[None, :]          # [n, 32]
    idx = (np.arange(P) % DH) // 2               # row -> freq index
    cos_t = np.cos(ang)[:, idx].T.astype(bf)     # [128, n]
    sin_t = np.sin(ang)[:, idx].T.astype(bf)

    PT = np.zeros((DH, DH), dtype=np.float32)
    for i in range(DH // 2):
        PT[2 * i + 1, 2 * i] = -1.0
        PT[2 * i, 2 * i + 1] = 1.0
    pswapT = np.zeros((P, P), dtype=np.float32)
    pswapT[0:DH, 0:DH] = PT
    pswapT[DH:P, DH:P] = PT
    pswapT = pswapT.astype(bf)

    ones_col = np.ones((P, 1), dtype=bf)
    ones_rowb = np.ones((1, P), dtype=bf)

    in_maps = []
    for c in range(NCORES):
        bi, hg = divmod(c, 4)
        hs = hg * NH
        xT = np.ascontiguousarray(x[bi, :n].T).astype(bf)
        wq = w_qkv[:, hs * DH:(hs + NH) * DH]
        wk = w_qkv[:, HEADS * DH + hs * DH:HEADS * DH + (hs + NH) * DH]
        wv = w_qkv[:, 2 * HEADS * DH + hs * DH:2 * HEADS * DH + (hs + NH) * DH]
        wg = w_gates[:, hs:hs + NH]
        w_qkg = (np.concatenate([wq, wk, wg], axis=1)
                 * gvec[:, None]).astype(bf)
        w_vp = np.zeros((DIM, NH * 65), dtype=np.float32)
        for h in range(NH):
            w_vp[:, h * 65:h * 65 + DH] = wv[:, h * DH:(h + 1) * DH]
        w_vp = (w_vp * gvec[:, None]).astype(bf)
        w_out_s = w_out[hs * DH:(hs + NH) * DH, :].astype(bf)
        # halved: the kernel computes sigmoid(z+b) as 0.5*tanh((z+b)/2)+0.5
        bgT = (b_gates[hs:hs + NH] / 2.0).reshape(NH, 1).astype(np.float32)
        in_maps.append({
            "xT": xT, "w_qkg": w_qkg, "w_vp": w_vp, "w_out_s": w_out_s,
            "cos_t": cos_t, "sin_t": sin_t, "pswapT": pswapT,
            "ones_col": ones_col, "ones_rowb": ones_rowb, "bgT": bgT,
        })
    return in_maps


_NC_CACHE = {}


def _ensure_ntff_hook():
    """antenv.axon_hooks is missing on this image; recreate it and register
    the ctypes NTFF profiling hook from trn_agent_boot so trace=True works."""
    try:
        from antenv.axon_hooks import get_axon_ntff_profile_hook  # noqa: F401
        return
    except ImportError:
        pass
    import types
    try:
        import antenv
    except ImportError:
        return
    mod = types.ModuleType("antenv.axon_hooks")
    holder = {}
    mod.set_axon_ntff_profile_hook = lambda h: holder.__setitem__("h", h)
    mod.get_axon_ntff_profile_hook = lambda: holder.get("h")
    sys.modules["antenv.axon_hooks"] = mod
    antenv.axon_hooks = mod
    try:
        from trn_agent_boot.trn_boot import _ntff_profile_via_ctypes
        h = _ntff_profile_via_ctypes("/opt/axon/libaxon_pjrt.so")
        if h is not None:
            mod.set_axon_ntff_profile_hook(h)
    except Exception:
        pass


def run(inputs, trace=False, n=N, dbg=False):
    if trace:
        _ensure_ntff_hook()
    key = (n, dbg)
    if key not in _NC_CACHE:
        _NC_CACHE[key] = build_graph(n, dbg=dbg)
    nc = _NC_CACHE[key]
    in_maps = host_prep(**inputs, n=n)
    kw = {}
    if trace:
        kw = dict(trace=True, trace_cores=[0])
    res = run_bass_kernel_spmd(nc, in_maps, core_ids=list(range(NCORES)), **kw)
    parts = [np.asarray(r["out"], dtype=np.float32) for r in res.results]
    out = np.stack([
        parts[0] + parts[1] + parts[2] + parts[3],
        parts[4] + parts[5] + parts[6] + parts[7],
    ]).astype(np.float32)
    return out, res


def kernel(**inputs):
    out, _ = run(inputs, trace=False)
    return out

